# revision 10
# baseline (speedup 1.0000x reference)
"""Multi-head attention (lazy K/V projections) Trainium2 Bass kernel, v2.

Problem: nn_MultiHeadAttention_54520314856024
  B=8, SQ=SK=1024, D=1024, E=128, H=32
  keys  = einsum('bsd,hde->hbse', states, Wk) + bk
  vals  = einsum('bsd,hde->hbse', states, Wv) + bv
  attn  = softmax(einsum('bqe,hbke->hbqk', query, keys) / sqrt(E))
  ctx   = einsum('hbqk,hbke->hbqe', attn, vals) -> concat heads -> @ Wc + bc

Sharding: batch-parallel, one batch element per NeuronCore (8 cores).

v2 changes vs v1:
  - bk dropped on device (softmax is shift-invariant in k: scores change by a
    per-(q,h) constant q.bk which cancels in the normalized attention).
  - bv/bc folded on host: sum_k attn = 1, so the bv contribution to the
    output is the constant sum_h bv_h @ Wc_h; merged into one bias vector.
  - software-pipelined emission: each head's PE stream interleaves the next
    head's keys projection and the next group's vals projection between the
    scores -> denom -> ctx chains, so PE never waits on ACT/DVE and stays at
    full clock.
  - PSUM pools double-buffered (scores/ctx) so consecutive heads overlap.

Per-head algebra (everything transposed, contractions on the partition axis):
  keysT_h [E, SK]  = Wk_h^T @ statesT            (PE)
  vals    [SK, GE] = statesT^T @ Wv_group        (PE, G=4 heads per group)
  scoresT [SK, SQ] = keysT^T @ queryT            (PE)
  exp     = exp(scoresT / sqrt(E))               (ACT)
  denom   [P, SQ]  = ones^T @ exp                (PE partition-reduce+broadcast)
  ctxnT   [E, SQ]  = (vals^T @ exp) * 1/denom    (PE + DVE)
  finalT  [E, SQ] += Wc_h^T @ ctxnT_h            (PE + DVE accumulate)
  out     [SQ, E]  = transpose(finalT) + bc_eff  (PE transpose)
"""

import sys

for _p in ("/opt/trn_rl_repo",):
    if _p not in sys.path:
        sys.path.insert(0, _p)

from contextlib import ExitStack, nullcontext

import numpy as np

import concourse.bass as bass
import concourse.mybir as mybir
import concourse.tile as tile
from concourse import bacc, bass_utils
from concourse.masks import make_identity

B, SQ, SK = 8, 1024, 1024
D, E, H = 1024, 128, 32
P = 128          # partition width
DCH = D // P     # 8 d-chunks
KT = SK // P     # 8 k-tiles
G = 4            # heads per vals-group
NG = H // G      # 8 groups
NH = 512         # matmul moving-dim chunk (fp32 max)
SCALE = 1.0 / float(np.sqrt(E))

F32 = mybir.dt.float32
F32R = mybir.dt.float32r
BF16 = mybir.dt.bfloat16

N_CORES = 8

import ml_dtypes

BF16_NP = np.dtype(ml_dtypes.bfloat16)

_COMPILED = {}
_ONES_SQ = np.ones((P, P), np.float32)


def build_nc(mm_dtype="bf16", n_iters=1, bodies=1):
    """Build the single-core Bass program (SPMD across 8 cores).

    n_iters > 1 wraps the whole body in a hardware loop that repeats the
    full computation (including input DMA) — used only for steady-state
    timing; kernel() always uses n_iters=1.
    """
    MT = {"bf16": BF16, "f32r": F32R, "f32": F32}[mm_dtype]

    nc = bacc.Bacc("TRN2", target_bir_lowering=False, debug=False)

    statesT = nc.dram_tensor("statesT", [D, SK], MT, kind="ExternalInput").ap()
    queryT = nc.dram_tensor("queryT", [E, SQ], MT, kind="ExternalInput").ap()
    WkT = nc.dram_tensor("WkT", [D, H * E], MT, kind="ExternalInput").ap()
    WvT = nc.dram_tensor("WvT", [D, H * E], MT, kind="ExternalInput").ap()
    Wc = nc.dram_tensor("Wc", [H * E, E], MT, kind="ExternalInput").ap()
    bcE = nc.dram_tensor("bcE", [E, 1], F32, kind="ExternalInput").ap()
    onesSQ = nc.dram_tensor("onesSQ", [P, P], F32R, kind="ExternalInput").ap()
    out = nc.dram_tensor("out", [SQ, E], F32, kind="ExternalOutput").ap()

    Wc3 = Wc.rearrange("(h e) f -> h e f", e=P)

    with tile.TileContext(nc) as tc, ExitStack() as es:
        constp = es.enter_context(tc.tile_pool(name="const", bufs=1))
        statesp = es.enter_context(tc.tile_pool(name="states", bufs=DCH))
        queryp = es.enter_context(tc.tile_pool(name="query", bufs=1))
        wkp = es.enter_context(tc.tile_pool(name="wk", bufs=16))
        wvp = es.enter_context(tc.tile_pool(name="wv", bufs=16))
        wcp = es.enter_context(tc.tile_pool(name="wc", bufs=8))
        keysp = es.enter_context(tc.tile_pool(name="keys", bufs=2))
        expp = es.enter_context(tc.tile_pool(name="exps", bufs=20))
        valsp = es.enter_context(tc.tile_pool(name="vals", bufs=16))
        recipp = es.enter_context(tc.tile_pool(name="recip", bufs=2))
        esump = es.enter_context(tc.tile_pool(name="esum", bufs=10))
        ctxp = es.enter_context(tc.tile_pool(name="ctx", bufs=2))
        finalp = es.enter_context(tc.tile_pool(name="final", bufs=1))
        outp = es.enter_context(tc.tile_pool(name="outs", bufs=KT))
        ps_score = es.enter_context(tc.tile_pool(name="ps_score", bufs=2, space="PSUM"))
        ps_den = es.enter_context(tc.tile_pool(name="ps_den", bufs=2, space="PSUM"))
        ps_ctx = es.enter_context(tc.tile_pool(name="ps_ctx", bufs=2, space="PSUM"))
        ps_kv = es.enter_context(tc.tile_pool(name="ps_kv", bufs=2, space="PSUM"))

        with tc.For_i(0, n_iters) if n_iters > 1 else nullcontext():
          for _rep in range(bodies):
            # ---- constants ----
            ones_sq = constp.tile([P, P], F32R)
            nc.sync.dma_start(ones_sq[:], onesSQ[:])
            ident = constp.tile([P, P], F32)
            make_identity(nc, ident[:])
            bc_t = constp.tile([E, 1], F32)
            nc.sync.dma_start(bc_t[:], bcE[:])

            # ---- resident activations ----
            st = []
            for d in range(DCH):
                st_t = statesp.tile([P, SK], MT)
                nc.sync.dma_start(st_t[:], statesT[d * P : (d + 1) * P, :])
                st.append(st_t)
            q_t = queryp.tile([E, SQ], MT)
            nc.sync.dma_start(q_t[:], queryT[:])

            final_t = finalp.tile([E, SQ], F32)

            # weight DMA for a group: 8 wv tiles, 8 wk tiles, 4 wc tiles
            wk_tiles = {}
            wv_tiles = {}
            wc_tiles = {}

            def fetch_group(g):
                if g >= NG:
                    return
                wv_tiles[g] = []
                wk_tiles[g] = []
                for d in range(DCH):
                    wv_t = wvp.tile([P, G * E], MT)
                    nc.sync.dma_start(
                        wv_t[:],
                        WvT[d * P : (d + 1) * P, g * G * E : (g + 1) * G * E],
                    )
                    wv_tiles[g].append(wv_t)
                for d in range(DCH):
                    wk_t = wkp.tile([P, G * E], MT)
                    nc.sync.dma_start(
                        wk_t[:],
                        WkT[d * P : (d + 1) * P, g * G * E : (g + 1) * G * E],
                    )
                    wk_tiles[g].append(wk_t)
                wc_tiles[g] = []
                for hg in range(G):
                    wc_t = wcp.tile([P, P], MT)
                    nc.sync.dma_start(wc_t[:], Wc3[g * G + hg])
                    wc_tiles[g].append(wc_t)

            vals_tiles = {}  # g -> list of 8 [P, G*E] SBUF tiles

            def vals_chunk(g, kts):
                """PE+DVE: compute vals[kt] for kt in kts for group g."""
                if g >= NG:
                    return
                lst = vals_tiles.setdefault(g, [None] * KT)
                for kt in kts:
                    pv = ps_kv.tile([P, G * E], F32, tag="kv")
                    for d in range(DCH):
                        nc.tensor.matmul(
                            pv[:],
                            st[d][:, kt * P : (kt + 1) * P],
                            wv_tiles[g][d][:],
                            start=(d == 0),
                            stop=(d == DCH - 1),
                        )
                    v_sb = valsp.tile([P, G * E], MT)
                    nc.vector.tensor_copy(v_sb[:], pv[:])
                    lst[kt] = v_sb

            keys_sb = {}  # h -> [E, SK] SBUF tile

            def keys_head(h):
                """PE+DVE: keysT for head h."""
                if h >= H:
                    return
                g, hg = divmod(h, G)
                k_sb = keysp.tile([E, SK], MT)
                for half in range(2):
                    pk = ps_kv.tile([P, NH], F32, tag="kv")
                    for d in range(DCH):
                        nc.tensor.matmul(
                            pk[:],
                            wk_tiles[g][d][:, hg * E : (hg + 1) * E],
                            st[d][:, half * NH : (half + 1) * NH],
                            start=(d == 0),
                            stop=(d == DCH - 1),
                        )
                    nc.vector.tensor_copy(k_sb[:, half * NH : (half + 1) * NH], pk[:])
                keys_sb[h] = k_sb


            # state carried between pipeline phases
            exp_tiles = {}   # h -> [16] exp tiles ([P, NH], kt-major within qh)
            recip_t = {}     # h -> [2] recip tiles [P, NH]
            ctxn_t = {}      # h -> [2] normalized ctx tiles [E, NH]

            exp_leaf = {}  # h -> [2][4] leaf partial sums ([P, NH] f32r)

            def emit_scores(h):
                if h >= H:
                    return
                tiles = [[None] * KT for _ in range(2)]
                leaves = [[], []]
                for kt in range(KT):
                    for qh in range(2):
                        ps = ps_score.tile([P, NH], F32, tag="score")
                        nc.tensor.matmul(
                            ps[:],
                            keys_sb[h][:, kt * P : (kt + 1) * P],
                            q_t[:, qh * NH : (qh + 1) * NH],
                            start=True,
                            stop=True,
                        )
                        ex = expp.tile([P, NH], MT)
                        nc.scalar.activation(
                            ex[:], ps[:], mybir.ActivationFunctionType.Exp,
                            scale=SCALE,
                        )
                        tiles[qh][kt] = ex
                        if kt % 2 == 1:
                            # leaf pre-sum of this exp pair (f32 accumulation)
                            lf = esump.tile([P, NH], F32R)
                            eng = nc.vector if (kt // 2) % 2 == 0 else nc.gpsimd
                            eng.tensor_add(
                                lf[:], tiles[qh][kt - 1][:], tiles[qh][kt][:]
                            )
                            leaves[qh].append(lf)
                exp_tiles[h] = tiles[0] + tiles[1]
                exp_leaf[h] = leaves

            def emit_denom(h):
                if h >= H:
                    return
                recs = []
                for qh in range(2):
                    l0, l1, l2, l3 = exp_leaf[h][qh]
                    m0 = esump.tile([P, NH], F32R)
                    nc.vector.tensor_add(m0[:], l0[:], l1[:])
                    m1 = esump.tile([P, NH], F32R)
                    nc.gpsimd.tensor_add(m1[:], l2[:], l3[:])
                    s = esump.tile([P, NH], F32R)
                    nc.vector.tensor_add(s[:], m0[:], m1[:])
                    pd = ps_den.tile([P, NH], F32, tag="den")
                    nc.tensor.matmul(
                        pd[:], ones_sq[:], s[:], start=True, stop=True
                    )
                    rec = recipp.tile([P, NH], F32)
                    nc.vector.reciprocal_approx_fast(out=rec[:], in_=pd[:])
                    recs.append(rec)
                recip_t[h] = recs

            def emit_ctx(h):
                if h >= H:
                    return
                g, hg = divmod(h, G)
                pcs = [
                    ps_ctx.tile([E, NH], F32, tag="ctx", name=f"pc{qh}")
                    for qh in range(2)
                ]
                for kt in range(KT):
                    for qh in range(2):
                        nc.tensor.matmul(
                            pcs[qh][:],
                            vals_tiles[g][kt][:, hg * E : (hg + 1) * E],
                            exp_tiles[h][qh * KT + kt][:],
                            start=(kt == 0),
                            stop=(kt == KT - 1),
                            skip_group_check=True,
                        )
                cts = []
                for qh in range(2):
                    ct = ctxp.tile([E, NH], MT)
                    nc.vector.tensor_mul(ct[:], pcs[qh][:], recip_t[h][qh][:])
                    cts.append(ct)
                ctxn_t[h] = cts
                # keys/exp tiles of head h are dead after this

            def emit_final(h):
                if h < 0 or h >= H:
                    return
                g, hg = divmod(h, G)
                for qh in range(2):
                    pf = ps_kv.tile([P, NH], F32, tag="kv")
                    nc.tensor.matmul(
                        pf[:],
                        wc_tiles[g][hg][:],
                        ctxn_t[h][qh][:],
                        start=True,
                        stop=True,
                    )
                    if h == 0:
                        nc.vector.tensor_scalar(
                            final_t[:, qh * NH : (qh + 1) * NH],
                            pf[:],
                            bc_t[:],
                            None,
                            op0=mybir.AluOpType.add,
                        )
                    else:
                        nc.vector.tensor_add(
                            final_t[:, qh * NH : (qh + 1) * NH],
                            final_t[:, qh * NH : (qh + 1) * NH],
                            pf[:],
                        )

            # ---- prologue: group 0 weights, vals(0), keys(0) ----
            fetch_group(0)
            fetch_group(1)
            vals_chunk(0, range(KT))
            keys_head(0)

            # ---- steady-state head pipeline ----
            for h in range(H):
                g, hg = divmod(h, G)
                if hg == 0 and h > 0:
                    fetch_group(g + 1)
                emit_scores(h)          # PE: 16 mm -> ACT exp
                keys_head(h + 1)        # PE: 16 mm (fills ACT exp latency)
                emit_denom(h)           # PE: 16 mm reading exp(h)
                emit_final(h - 1)       # PE: 2 mm (ctxn(h-1) long ready)
                vals_chunk(g + 1, (2 * hg, 2 * hg + 1))  # PE: 16 mm prefetch
                emit_ctx(h)             # PE: 16 mm reading exp(h)
            emit_final(H - 1)

            # ---- transpose finalT -> out [SQ, E] ----
            for qt in range(KT):
                pt = ps_kv.tile([P, P], F32, tag="kv")
                nc.tensor.transpose(
                    pt[:], final_t[:, qt * P : (qt + 1) * P], ident[:]
                )
                o_sb = outp.tile([P, E], F32)
                nc.vector.tensor_copy(o_sb[:], pt[:])
                nc.sync.dma_start(out[qt * P : (qt + 1) * P, :], o_sb[:])

    nc.compile()
    return nc


def _prep_inputs(query, states, Wk, bk, Wv, bv, Wc, bc):
    """Host-side sharding: per-core input maps (core c == batch element c).

    bk is dropped (softmax shift-invariance); bv and bc are folded into a
    single output bias  bc_eff = bc + sum_h bv_h @ Wc_h  (valid because the
    attention weights of each head sum to one).
    """
    query = np.asarray(query, np.float32)
    states = np.asarray(states, np.float32)
    Wk = np.asarray(Wk, np.float32)
    bv = np.asarray(bv, np.float32)
    Wv = np.asarray(Wv, np.float32)
    Wc = np.asarray(Wc, np.float32)
    bc = np.asarray(bc, np.float32)

    WkT = np.ascontiguousarray(Wk.transpose(1, 0, 2).reshape(D, H * E)).astype(BF16_NP)
    WvT = np.ascontiguousarray(Wv.transpose(1, 0, 2).reshape(D, H * E)).astype(BF16_NP)
    WcC = np.ascontiguousarray(Wc).astype(BF16_NP)
    bc_eff = bc + bv.reshape(1, H * E) @ Wc
    bcE = np.ascontiguousarray(bc_eff.reshape(E, 1))

    in_maps = []
    for c in range(N_CORES):
        in_maps.append(
            {
                "statesT": np.ascontiguousarray(states[c].T).astype(BF16_NP),
                "queryT": np.ascontiguousarray(query[c].T).astype(BF16_NP),
                "WkT": WkT,
                "WvT": WvT,
                "Wc": WcC,
                "bcE": bcE,
                "onesSQ": _ONES_SQ,
            }
        )
    return in_maps


def get_nc(mm_dtype="bf16", n_iters=1, bodies=1):
    key = (mm_dtype, n_iters, bodies)
    nc = _COMPILED.get(key)
    if nc is None:
        nc = build_nc(mm_dtype, n_iters=n_iters, bodies=bodies)
        _COMPILED[key] = nc
    return nc


def kernel(query, states, Wk, bk, Wv, bv, Wc, bc):
    nc = get_nc()
    in_maps = _prep_inputs(query, states, Wk, bk, Wv, bv, Wc, bc)
    res = bass_utils.run_bass_kernel_spmd(nc, in_maps, list(range(N_CORES)))
    return np.stack([res.results[c]["out"] for c in range(N_CORES)], axis=0)
